# revision 1
# baseline (speedup 1.0000x reference)
"""Neural CDE kernel for Trainium2 (8 NeuronCores, data-parallel over batch).

Problem shapes (hardcoded per contract): B=512, T=1024, D=8, H=64, W=128.

Host side: knot index / frac from ts (exact fp32 accumulation semantics),
spline derivative dX, initial MLP y0, and folding of dt plus the
tanh(z) = 1 - 2*sigmoid(-2z) rewrite into a padded dX9 tensor.

Device side (per core, 64 samples, scan fully unrolled):
  p1 = Wf0 @ y            (PE, weight-stationary)
  h1 = ln(1 + exp(p1+b0)) (ACT Exp + Ln(bias=1))   [natural_log_exp set]
  p2 = Wf1 @ h1           (PE)
  h2 = ln(1 + exp(p2+b1)) (ACT)
  z  = Wf2 @ h2 + b2      (PE, data-stationary, + K=1 ones-matmul for bias)
  S  = sigmoid(-2z) = exp(-ln(1+exp(2z)))          (ACT x3)
  q[s,h] = sum_d S9[s,(h,d)] * dX9[s,k,d]          (DVE mul + grouped reduce)
           where S9 has a constant-1 column at d=8 and
           dX9[...,d<8] = -2*dt*dX, dX9[...,8] = dt*sum_d dX
           => q = dt * sum_d tanh(z_d) * dX_d
  y += q^T                (PE transpose + DVE add)
  ro[:,k] = y^T @ Wl      (PE, N=1 matmul into accumulating PSUM bank)
Final: sigmoid via the same exp/ln chain, DMA out.
"""

import numpy as np

B, T, D, H, W = 512, 1024, 8, 64, 128
NCORES = 8
S = B // NCORES  # samples per core = 64
D9 = D + 1       # padded derivative cols


# ----------------------------------------------------------------- host math
def _host_precompute(ts, cd, cc, cb, ca, Wi0, bi0, Wi1, bi1, Wi2, bi2):
    f32 = np.float32
    ts = np.asarray(ts, f32)
    dt = (ts[:, 1] - ts[:, 0]).astype(f32)  # (B,)

    # t0 series: t0_{k+1} = t0_k + dt accumulated in fp32 (cumsum is sequential)
    incs = np.concatenate([ts[:, :1], np.tile(dt[:, None], (1, T - 1))], axis=1)
    t0 = np.cumsum(incs, axis=1, dtype=f32)  # (B, T)

    # knot index + frac per row (searchsorted 'right' like the oracle)
    idx = np.empty((B, T), np.int64)
    for b in range(B):
        idx[b] = np.searchsorted(ts[b], t0[b], side="right") - 1
    idx = np.clip(idx, 0, T - 2)
    frac = (t0 - np.take_along_axis(ts, idx, axis=1)).astype(f32)  # (B, T)

    rows = np.arange(B)[:, None]
    cbg = cb[rows, idx]  # (B, T, D)
    ccg = cc[rows, idx]
    cdg = cd[rows, idx]
    fr = frac[:, :, None]
    dX = (cbg + fr * (f32(2.0) * ccg + f32(3.0) * fr * cdg)).astype(f32)

    dtb = dt[:, None, None]
    dX9 = np.empty((B, T, D9), f32)
    dX9[:, :, :D] = f32(-2.0) * dtb * dX
    dX9[:, :, D] = (dtb[:, :, 0] * dX.sum(axis=2)).astype(f32)

    # initial MLP (relu hidden): y0 = Wi2 @ relu(Wi1 @ relu(Wi0 @ a0 + bi0) + bi1) + bi2
    a0 = np.asarray(ca, f32)[:, 0, :]  # (B, D)
    hh = np.maximum(a0 @ np.asarray(Wi0, f32).T + bi0, 0)
    hh = np.maximum(hh @ np.asarray(Wi1, f32).T + bi1, 0)
    y0 = (hh @ np.asarray(Wi2, f32).T + bi2).astype(f32)  # (B, H)
    return dX9, y0


# --------------------------------------------------------------- bass kernel
def _build_kernel(bl_val):
    import concourse.bass as bass
    import concourse.bacc as bacc
    import concourse.mybir as mybir
    from concourse.tile import TileContext

    f32 = mybir.dt.float32
    bf16 = mybir.dt.bfloat16
    AF = mybir.ActivationFunctionType
    ALU = mybir.AluOpType

    nc = bacc.Bacc("TRN2")

    # DRAM I/O (per-core shapes)
    d_w0t = nc.dram_tensor("w0t", [H, W], f32, kind="ExternalInput")      # Wf0^T
    d_w1t = nc.dram_tensor("w1t", [W, W], f32, kind="ExternalInput")      # Wf1^T
    d_w2t = nc.dram_tensor("w2t", [W, H * D], f32, kind="ExternalInput")  # Wf2^T
    d_wlt = nc.dram_tensor("wlt", [H, 1], f32, kind="ExternalInput")      # Wl^T
    d_b0 = nc.dram_tensor("b0", [W, 1], f32, kind="ExternalInput")
    d_b1 = nc.dram_tensor("b1", [W, 1], f32, kind="ExternalInput")
    d_b2 = nc.dram_tensor("b2", [1, H * D], f32, kind="ExternalInput")
    d_ones = nc.dram_tensor("ones1", [1, S], f32, kind="ExternalInput")
    d_ident = nc.dram_tensor("ident", [S, S], f32, kind="ExternalInput")
    d_dx9 = nc.dram_tensor("dx9", [S, T * D9], bf16, kind="ExternalInput")
    d_y0t = nc.dram_tensor("y0t", [H, S], f32, kind="ExternalInput")
    d_out = nc.dram_tensor("out", [S, T], f32, kind="ExternalOutput")

    UNROLL = 16
    assert T % UNROLL == 0

    with TileContext(nc) as tc:
        with (
            tc.tile_pool(name="const", bufs=1) as cpool,
            tc.tile_pool(name="state", bufs=1) as spool,
            tc.tile_pool(name="work", bufs=2) as wpool,
            tc.tile_pool(name="ps", bufs=2, space="PSUM") as ppool,
            tc.tile_pool(name="ps1", bufs=1, space="PSUM") as p1pool,
        ):
            # constants
            w0t = cpool.tile([H, W], f32, tag="w0t")
            w1t = cpool.tile([W, W], f32, tag="w1t")
            w2t = cpool.tile([W, H * D], f32, tag="w2t")
            wlt = cpool.tile([H, 1], f32, tag="wlt")
            b0 = cpool.tile([W, 1], f32, tag="b0")
            b1 = cpool.tile([W, 1], f32, tag="b1")
            b2 = cpool.tile([1, H * D], f32, tag="b2")
            ones1 = cpool.tile([1, S], f32, tag="ones1")
            ident = cpool.tile([S, S], f32, tag="ident")
            dx9 = cpool.tile([S, T * D9], bf16, tag="dx9")
            for dst, src in [
                (w0t, d_w0t), (w1t, d_w1t), (w2t, d_w2t), (wlt, d_wlt),
                (b0, d_b0), (b1, d_b1), (b2, d_b2), (ones1, d_ones),
                (ident, d_ident), (dx9, d_dx9),
            ]:
                nc.gpsimd.dma_start(dst[:], src[:])

            # state
            y = spool.tile([H, S], f32, tag="y")  # (h, s)
            nc.gpsimd.dma_start(y[:], d_y0t[:])
            # S9 double buffer, const-1 column at d=8
            s9 = [
                spool.tile([S, H * D9], bf16, tag=f"s9_{i}", name=f"s9_{i}")
                for i in range(2)
            ]
            for t_ in s9:
                v = t_[:].rearrange("s (h d) -> s h d", d=D9)
                nc.vector.memset(v[:, :, D : D + 1], 1.0)

            ro_sb = spool.tile([S, T], f32, tag="ro_sb")
            ro_ps = p1pool.tile([S, UNROLL], f32, tag="ro_ps")

            # Constants settle before any compute touches them: a matmul
            # (S3_LW struct) cannot carry multiple HWDGE sem waits.
            tc.strict_bb_all_engine_barrier()

            with tc.For_i(0, T // UNROLL, 1) as iv:
              ibase = iv * (UNROLL * D9)
              for j in range(UNROLL):
                k = j  # static within the unrolled body
                s9k = s9[k % 2]
                # ---- mm1: p1 = Wf0 @ y  -> (W, S)
                p1 = ppool.tile([W, S], f32, tag="p12")
                nc.tensor.matmul(p1[:], w0t[:], y[:], start=True, stop=True)
                # ---- softplus 1 (with bias b0 folded into Exp)
                u1 = wpool.tile([W, S], f32, tag="u1")
                h1 = wpool.tile([W, S], f32, tag="h1")
                nc.scalar.activation(u1[:], p1[:], AF.Exp, bias=b0[:])
                nc.scalar.activation(h1[:], u1[:], AF.Ln, bias=1.0)
                # ---- mm2
                p2 = ppool.tile([W, S], f32, tag="p12")
                nc.tensor.matmul(p2[:], w1t[:], h1[:], start=True, stop=True)
                u2 = wpool.tile([W, S], f32, tag="u2")
                h2 = wpool.tile([W, S], f32, tag="h2")
                nc.scalar.activation(u2[:], p2[:], AF.Exp, bias=b1[:])
                nc.scalar.activation(h2[:], u2[:], AF.Ln, bias=1.0)
                # ---- mm3: z = h2^T W2T + b2 -> (S, H*D)
                vf = ppool.tile([S, H * D], f32, tag="vf")
                nc.tensor.matmul(vf[:], ones1[:], b2[:], start=True, stop=False)
                nc.tensor.matmul(vf[:], h2[:], w2t[:], start=False, stop=True)
                # ---- S = sigmoid(-2z) = exp(-ln(1+exp(2z)))
                e2 = wpool.tile([S, H * D], f32, tag="e2")
                l2 = wpool.tile([S, H * D], f32, tag="l2")
                nc.scalar.activation(e2[:], vf[:], AF.Exp, scale=2.0)
                nc.scalar.activation(l2[:], e2[:], AF.Ln, bias=1.0)
                s9v = s9k[:].rearrange("s (h d) -> s h d", d=D9)
                l2v = l2[:].rearrange("s (h d) -> s h d", d=D)
                nc.scalar.activation(s9v[:, :, 0:D], l2v, AF.Exp, scale=-1.0)
                # ---- q[s,h] = sum_d S9 * dX9  (broadcast dx over h)
                m1 = wpool.tile([S, H * D9], bf16, tag="m1")
                dxk = dx9[:, bass.ds(ibase + j * D9, D9)]
                dxb = dxk.rearrange("s (o d) -> s o d", o=1)
                m1v = m1[:].rearrange("s (h d) -> s h d", d=D9)
                s9vv = s9k[:].rearrange("s (h d) -> s h d", d=D9)
                in0b, in1b = bass.broadcast_tensor_aps(s9vv, dxb)
                nc.vector.tensor_tensor(m1v, in0b, in1b, ALU.mult)
                q = wpool.tile([S, H], f32, tag="q")
                nc.vector.tensor_reduce(
                    q[:], m1v, axis=mybir.AxisListType.X, op=ALU.add
                )
                # ---- y += q^T
                qt = ppool.tile([H, S], f32, tag="qt")
                nc.tensor.transpose(qt[:], q[:], ident[:])
                nc.vector.tensor_tensor(y[:], y[:], qt[:], ALU.add)
                # ---- readout column
                nc.tensor.matmul(
                    ro_ps[:, j : j + 1], y[:], wlt[:], start=True, stop=True
                )
                if j == UNROLL - 1:
                    nc.vector.tensor_copy(
                        ro_sb[:, bass.ds(iv * UNROLL, UNROLL)], ro_ps[:]
                    )

            # ---- final sigmoid(v + bl) = exp(-ln(1+exp(-v-bl)))
            eo = spool.tile([S, T], f32, tag="eo")
            nc.scalar.activation(eo[:], ro_sb[:], AF.Exp, scale=-1.0,
                                 bias=float(-bl_val))
            nc.scalar.activation(eo[:], eo[:], AF.Ln, bias=1.0)
            nc.scalar.activation(eo[:], eo[:], AF.Exp, scale=-1.0)
            nc.sync.dma_start(d_out[:], eo[:])

    nc.compile()
    return nc


_NC_CACHE = {}
LAST_RESULTS = None


def _get_nc(bl_val):
    key = float(bl_val)
    if key not in _NC_CACHE:
        _NC_CACHE[key] = _build_kernel(key)
    return _NC_CACHE[key]


# ------------------------------------------------------------------- driver
def kernel(ts, cd, cc, cb, ca, Wi0, bi0, Wi1, bi1, Wi2, bi2,
           Wf0, bf0, Wf1, bf1, Wf2, bf2, Wl, bl):
    import ml_dtypes
    from concourse.bass_utils import run_bass_kernel_spmd

    f32 = np.float32
    ts, cd, cc, cb, ca = (np.asarray(x, f32) for x in (ts, cd, cc, cb, ca))
    dX9, y0 = _host_precompute(ts, cd, cc, cb, ca, Wi0, bi0, Wi1, bi1, Wi2, bi2)

    Wf0, Wf1, Wf2, Wl = (np.asarray(x, f32) for x in (Wf0, Wf1, Wf2, Wl))
    bf0, bf1, bf2, bl = (np.asarray(x, f32) for x in (bf0, bf1, bf2, bl))

    shared = {
        "w0t": np.ascontiguousarray(Wf0.T),              # (H, W)
        "w1t": np.ascontiguousarray(Wf1.T),              # (W, W)
        "w2t": np.ascontiguousarray(Wf2.T),              # (W, H*D)
        "wlt": np.ascontiguousarray(Wl[0][:, None]),     # (H, 1)
        "b0": np.ascontiguousarray(bf0[:, None]),
        "b1": np.ascontiguousarray(bf1[:, None]),
        "b2": np.ascontiguousarray(bf2[None, :]),
        "ones1": np.ones((1, S), f32),
        "ident": np.eye(S, dtype=f32),
    }

    nc = _get_nc(float(bl[0]))
    in_maps = []
    for c in range(NCORES):
        sl = slice(c * S, (c + 1) * S)
        m = dict(shared)
        m["dx9"] = np.ascontiguousarray(
            dX9[sl].reshape(S, T * D9)).astype(ml_dtypes.bfloat16)
        m["y0t"] = np.ascontiguousarray(y0[sl].T)        # (H, S)
        in_maps.append(m)

    res = run_bass_kernel_spmd(nc, in_maps, core_ids=list(range(NCORES)))
    global LAST_RESULTS
    LAST_RESULTS = res
    out = np.concatenate([res.results[c]["out"] for c in range(NCORES)], axis=0)
    return out.astype(f32)



# revision 4
# speedup vs baseline: 3.5717x; 3.5717x over previous
"""Neural CDE kernel for Trainium2 (8 NeuronCores, data-parallel over batch).

Problem shapes (hardcoded per contract): B=512, T=1024, D=8, H=64, W=128.

Host side (fast path, ts rows identical as produced by setup_inputs):
knot index / frac from ts row 0 (exact fp32 accumulation semantics), then a
jax-CPU jitted fused pass builds the quantized spline-derivative tensor and
the initial-MLP state:
  dxq9[b,k,d<8] = e4m3(-2*C*dt*dX[b,k,d]),  dxq9[b,k,8] = e4m3(C*dt*sum_d dX)
  with C = 2**14; the 1/C descale is folded into the transpose identity
  matrix shipped to the device (ident = I/C), so on device
  q = (1/C) * [sum_d S_d * (-2*C*dt*dX_d) + 1 * (C*dt*sum dX)]
    = dt * sum_d tanh(z_d) * dX_d        (tanh(z) = 1 - 2*sigmoid(-2z)).

Device side (per core, 64 samples, scan fully unrolled; all activations use
the natural_log_exp ACT table -- no 1.3us table reloads):
  p1 = Wf0 @ y            (PE, weight-stationary)
  h1 = ln(1 + exp(p1+b0)) (ACT Exp + Ln(bias=1))
  p2 = Wf1 @ h1           (PE)
  h2 = ln(1 + exp(p2+b1)) (ACT)
  z  = Wf2 @ h2 + b2      (PE, data-stationary, + K=1 ones-matmul for bias)
  S  = sigmoid(-2z) = exp(-ln(1+exp(2z)))          (ACT x3)
  q[s,h] = sum_d S9[s,(h,d)] * dxq9[s,k,d]         (DVE mul + grouped reduce)
           where S9 has a constant-1 column at d=8
  y += (q^T)/C            (PE transpose via scaled identity + DVE add)
  ro[:,k] = y^T @ Wl      (PE, N=1 matmul into accumulating PSUM bank)
Final: sigmoid via the same exp/ln chain, last Exp emits bf16, DMA out.

Dispatch: the shard_map jit, the compiled NEFF, and the device-resident
replicated weights are cached across calls (weights re-verified by hash each
call); per call only dxq9 (fp8, 4.7MB) and y0t (128KB) are transferred, the
previous call's output buffer is donated as the new output allocation, and
the bf16 output (1MB) is fetched back.
"""

import hashlib

import numpy as np

B, T, D, H, W = 512, 1024, 8, 64, 128
NCORES = 8
S = B // NCORES  # samples per core = 64
D9 = D + 1       # padded derivative cols
C_SCALE = float(2 ** 14)  # fp8 pre-scale; descale folded into ident input


# ----------------------------------------------------------------- host math
_FUSED_JIT = None


def _get_fused_jit():
    """jax-CPU jitted gather+FMA+quantize+init-MLP (single fused pass)."""
    global _FUSED_JIT
    if _FUSED_JIT is None:
        import jax
        import jax.numpy as jnp

        cpu = jax.devices("cpu")[0]

        def _fused(cb, cc, cd, idx0, frac0, dtv, ca0,
                   Wi0, bi0, Wi1, bi1, Wi2, bi2):
            cbg = cb[:, idx0]
            ccg = cc[:, idx0]
            cdg = cd[:, idx0]
            fr = frac0[None, :, None]
            dX = cbg + fr * (2.0 * ccg + 3.0 * fr * cdg)       # (B, T, D)
            a = (-2.0 * C_SCALE * dtv) * dX
            s = (C_SCALE * dtv) * dX.sum(axis=2, keepdims=True)
            dxq9 = jnp.concatenate([a, s], axis=2)
            dxq9 = dxq9.astype(jnp.float8_e4m3).reshape(B, T * D9)
            h = jax.nn.relu(ca0 @ Wi0.T + bi0)
            h = jax.nn.relu(h @ Wi1.T + bi1)
            y0 = h @ Wi2.T + bi2                               # (B, H)
            y0t = y0.reshape(NCORES, S, H).transpose(0, 2, 1).reshape(B, H)
            return dxq9, y0t

        jitted = jax.jit(_fused)

        def run(*a):
            with jax.default_device(cpu):
                return jitted(*a)

        _FUSED_JIT = run
    return _FUSED_JIT


def _host_precompute(ts, cd, cc, cb, ca, Wi0, bi0, Wi1, bi1, Wi2, bi2):
    """Returns (dxq9 (B, T*9) fp8e4m3, y0t (B, H) f32, ident_scale float).

    dxq9 rows are in batch order == concatenated per-core blocks; y0t is the
    per-core-transposed y0 ((core, H, S) flattened on axis 0).
    """
    f32 = np.float32
    ts = np.asarray(ts, f32)
    if bool((ts[1:] == ts[:1]).all()):
        # fast path: every row of ts identical (uniform grid from the oracle)
        dt = f32(ts[0, 1] - ts[0, 0])
        incs = np.concatenate([ts[0, :1], np.full(T - 1, dt, f32)])
        t0 = np.cumsum(incs, dtype=f32)
        idx0 = np.clip(np.searchsorted(ts[0], t0, side="right") - 1, 0, T - 2)
        frac0 = (t0 - ts[0][idx0]).astype(f32)
        fn = _get_fused_jit()
        dxq9, y0t = fn(
            np.asarray(cb, f32), np.asarray(cc, f32), np.asarray(cd, f32),
            idx0.astype(np.int32), frac0, dt,
            np.asarray(ca, f32)[:, 0, :],
            np.asarray(Wi0, f32), np.asarray(bi0, f32),
            np.asarray(Wi1, f32), np.asarray(bi1, f32),
            np.asarray(Wi2, f32), np.asarray(bi2, f32),
        )
        return np.asarray(dxq9), np.asarray(y0t), 1.0 / C_SCALE

    # general fallback (never taken for the oracle's inputs): per-row grids
    import ml_dtypes

    dt = (ts[:, 1] - ts[:, 0]).astype(f32)  # (B,)
    incs = np.concatenate([ts[:, :1], np.tile(dt[:, None], (1, T - 1))], axis=1)
    t0 = np.cumsum(incs, axis=1, dtype=f32)
    idx = np.empty((B, T), np.int64)
    for b in range(B):
        idx[b] = np.searchsorted(ts[b], t0[b], side="right") - 1
    idx = np.clip(idx, 0, T - 2)
    frac = (t0 - np.take_along_axis(ts, idx, axis=1)).astype(f32)
    rows = np.arange(B)[:, None]
    fr = frac[:, :, None]
    cb, cc, cd = (np.asarray(x, f32) for x in (cb, cc, cd))
    dX = (cb[rows, idx] + fr * (f32(2.0) * cc[rows, idx]
                                + f32(3.0) * fr * cd[rows, idx])).astype(f32)
    dtb = dt[:, None, None]
    a = f32(-2.0 * C_SCALE) * dtb * dX
    s = (f32(C_SCALE) * dtb[:, :, 0] * dX.sum(axis=2)).astype(f32)
    vmax = max(np.abs(a).max(), np.abs(s).max(), 1e-30)
    extra = 1.0
    while vmax / extra > 200.0:  # keep quantized values in e4m3 normal range
        extra *= 2.0
    dxq9 = np.empty((B, T, D9), ml_dtypes.float8_e4m3)
    dxq9[:, :, :D] = (a / f32(extra)).astype(ml_dtypes.float8_e4m3)
    dxq9[:, :, D] = (s / f32(extra)).astype(ml_dtypes.float8_e4m3)
    dxq9 = dxq9.reshape(B, T * D9)

    a0 = np.asarray(ca, f32)[:, 0, :]
    hh = np.maximum(a0 @ np.asarray(Wi0, f32).T + np.asarray(bi0, f32), 0)
    hh = np.maximum(hh @ np.asarray(Wi1, f32).T + np.asarray(bi1, f32), 0)
    y0 = (hh @ np.asarray(Wi2, f32).T + np.asarray(bi2, f32)).astype(f32)
    y0t = np.ascontiguousarray(
        y0.reshape(NCORES, S, H).transpose(0, 2, 1).reshape(B, H))
    return dxq9, y0t, extra / C_SCALE


# --------------------------------------------------------------- bass kernel
def _build_kernel(bl_val):
    import concourse.bass as bass
    import concourse.bacc as bacc
    import concourse.mybir as mybir
    from concourse.tile import TileContext

    f32 = mybir.dt.float32
    bf16 = mybir.dt.bfloat16
    fp8 = mybir.dt.float8e4
    AF = mybir.ActivationFunctionType
    ALU = mybir.AluOpType

    nc = bacc.Bacc("TRN2")

    # DRAM I/O (per-core shapes)
    d_w0t = nc.dram_tensor("w0t", [H, W], f32, kind="ExternalInput")      # Wf0^T
    d_w1t = nc.dram_tensor("w1t", [W, W], f32, kind="ExternalInput")      # Wf1^T
    d_w2t = nc.dram_tensor("w2t", [W, H * D], f32, kind="ExternalInput")  # Wf2^T
    d_wlt = nc.dram_tensor("wlt", [H, 1], f32, kind="ExternalInput")      # Wl^T
    d_b0 = nc.dram_tensor("b0", [W, 1], f32, kind="ExternalInput")
    d_b1 = nc.dram_tensor("b1", [W, 1], f32, kind="ExternalInput")
    d_b2 = nc.dram_tensor("b2", [1, H * D], f32, kind="ExternalInput")
    d_ones = nc.dram_tensor("ones1", [1, S], f32, kind="ExternalInput")
    d_ident = nc.dram_tensor("ident", [S, S], f32, kind="ExternalInput")  # I/C
    d_dx9 = nc.dram_tensor("dx9", [S, T * D9], fp8, kind="ExternalInput")
    d_y0t = nc.dram_tensor("y0t", [H, S], f32, kind="ExternalInput")
    d_out = nc.dram_tensor("out", [S, T], bf16, kind="ExternalOutput")

    UNROLL = 16
    assert T % UNROLL == 0

    with TileContext(nc) as tc:
        with (
            tc.tile_pool(name="const", bufs=1) as cpool,
            tc.tile_pool(name="state", bufs=1) as spool,
            tc.tile_pool(name="work", bufs=2) as wpool,
            tc.tile_pool(name="ps", bufs=2, space="PSUM") as ppool,
            tc.tile_pool(name="ps1", bufs=1, space="PSUM") as p1pool,
        ):
            # constants
            w0t = cpool.tile([H, W], f32, tag="w0t")
            w1t = cpool.tile([W, W], f32, tag="w1t")
            w2t = cpool.tile([W, H * D], f32, tag="w2t")
            wlt = cpool.tile([H, 1], f32, tag="wlt")
            b0 = cpool.tile([W, 1], f32, tag="b0")
            b1 = cpool.tile([W, 1], f32, tag="b1")
            b2 = cpool.tile([1, H * D], f32, tag="b2")
            ones1 = cpool.tile([1, S], f32, tag="ones1")
            ident = cpool.tile([S, S], f32, tag="ident")
            dx9q = cpool.tile([S, T * D9], fp8, tag="dx9q")
            for dst, src in [
                (w0t, d_w0t), (w1t, d_w1t), (w2t, d_w2t), (wlt, d_wlt),
                (b0, d_b0), (b1, d_b1), (b2, d_b2), (ones1, d_ones),
                (ident, d_ident), (dx9q, d_dx9),
            ]:
                nc.gpsimd.dma_start(dst[:], src[:])

            # fp8 -> bf16 bulk upcast (Copy is in every ACT table)
            dx9 = cpool.tile([S, T * D9], bf16, tag="dx9")
            nc.scalar.activation(dx9[:], dx9q[:], AF.Copy)

            # state
            y = spool.tile([H, S], f32, tag="y")  # (h, s)
            nc.gpsimd.dma_start(y[:], d_y0t[:])
            # S9 double buffer, const-1 column at d=8
            s9 = [
                spool.tile([S, H * D9], bf16, tag=f"s9_{i}", name=f"s9_{i}")
                for i in range(2)
            ]
            for t_ in s9:
                v = t_[:].rearrange("s (h d) -> s h d", d=D9)
                nc.vector.memset(v[:, :, D : D + 1], 1.0)

            ro_sb = spool.tile([S, T], f32, tag="ro_sb")
            ro_ps = p1pool.tile([S, UNROLL], f32, tag="ro_ps")

            # Constants settle before any compute touches them: a matmul
            # (S3_LW struct) cannot carry multiple HWDGE sem waits.
            tc.strict_bb_all_engine_barrier()

            with tc.For_i(0, T // UNROLL, 1) as iv:
              ibase = iv * (UNROLL * D9)
              for j in range(UNROLL):
                k = j  # static within the unrolled body
                s9k = s9[k % 2]
                # ---- mm1: p1 = Wf0 @ y  -> (W, S)
                p1 = ppool.tile([W, S], f32, tag="p12")
                nc.tensor.matmul(p1[:], w0t[:], y[:], start=True, stop=True)
                # ---- softplus 1 (with bias b0 folded into Exp)
                u1 = wpool.tile([W, S], f32, tag="u1")
                h1 = wpool.tile([W, S], f32, tag="h1")
                nc.scalar.activation(u1[:], p1[:], AF.Exp, bias=b0[:])
                nc.scalar.activation(h1[:], u1[:], AF.Ln, bias=1.0)
                # ---- mm2
                p2 = ppool.tile([W, S], f32, tag="p12")
                nc.tensor.matmul(p2[:], w1t[:], h1[:], start=True, stop=True)
                u2 = wpool.tile([W, S], f32, tag="u2")
                h2 = wpool.tile([W, S], f32, tag="h2")
                nc.scalar.activation(u2[:], p2[:], AF.Exp, bias=b1[:])
                nc.scalar.activation(h2[:], u2[:], AF.Ln, bias=1.0)
                # ---- mm3: z = h2^T W2T + b2 -> (S, H*D)
                vf = ppool.tile([S, H * D], f32, tag="vf")
                nc.tensor.matmul(vf[:], ones1[:], b2[:], start=True, stop=False)
                nc.tensor.matmul(vf[:], h2[:], w2t[:], start=False, stop=True)
                # ---- S = sigmoid(-2z) = exp(-ln(1+exp(2z)))
                e2 = wpool.tile([S, H * D], f32, tag="e2")
                l2 = wpool.tile([S, H * D], f32, tag="l2")
                nc.scalar.activation(e2[:], vf[:], AF.Exp, scale=2.0)
                nc.scalar.activation(l2[:], e2[:], AF.Ln, bias=1.0)
                s9v = s9k[:].rearrange("s (h d) -> s h d", d=D9)
                l2v = l2[:].rearrange("s (h d) -> s h d", d=D)
                nc.scalar.activation(s9v[:, :, 0:D], l2v, AF.Exp, scale=-1.0)
                # ---- q[s,h] = sum_d S9 * dxq9  (broadcast dx over h)
                m1 = wpool.tile([S, H * D9], bf16, tag="m1")
                dxk = dx9[:, bass.ds(ibase + j * D9, D9)]
                dxb = dxk.rearrange("s (o d) -> s o d", o=1)
                m1v = m1[:].rearrange("s (h d) -> s h d", d=D9)
                s9vv = s9k[:].rearrange("s (h d) -> s h d", d=D9)
                in0b, in1b = bass.broadcast_tensor_aps(s9vv, dxb)
                nc.vector.tensor_tensor(m1v, in0b, in1b, ALU.mult)
                q = wpool.tile([S, H], f32, tag="q")
                nc.vector.tensor_reduce(
                    q[:], m1v, axis=mybir.AxisListType.X, op=ALU.add
                )
                # ---- y += (q^T)/C: real matmul q^T @ (I/C) — transpose mode
                # would ignore the identity's values, dropping the descale
                qt = ppool.tile([H, S], f32, tag="qt")
                nc.tensor.matmul(qt[:], q[:], ident[:], start=True, stop=True)
                nc.vector.tensor_tensor(y[:], y[:], qt[:], ALU.add)
                # ---- readout column
                nc.tensor.matmul(
                    ro_ps[:, j : j + 1], y[:], wlt[:], start=True, stop=True
                )
                if j == UNROLL - 1:
                    nc.vector.tensor_copy(
                        ro_sb[:, bass.ds(iv * UNROLL, UNROLL)], ro_ps[:]
                    )

            # ---- final sigmoid(v + bl) = exp(-ln(1+exp(-v-bl)))
            eo = spool.tile([S, T], f32, tag="eo")
            eo16 = spool.tile([S, T], bf16, tag="eo16")
            nc.scalar.activation(eo[:], ro_sb[:], AF.Exp, scale=-1.0,
                                 bias=float(-bl_val))
            nc.scalar.activation(eo[:], eo[:], AF.Ln, bias=1.0)
            nc.scalar.activation(eo16[:], eo[:], AF.Exp, scale=-1.0)
            nc.sync.dma_start(d_out[:], eo16[:])

    nc.compile()
    return nc


# ------------------------------------------------------------------ dispatch
_STATE = None
LAST_RESULTS = None  # kept for test harness compatibility (always None)


def _get_state(bl_val):
    """Build-once state: bass module, shard_map jit, mesh, name order."""
    global _STATE
    if _STATE is not None and _STATE["bl_val"] == float(bl_val):
        return _STATE

    import jax
    from jax.sharding import Mesh, NamedSharding, PartitionSpec
    from jax.experimental.shard_map import shard_map
    import concourse.mybir as mybir
    from concourse.bass2jax import (
        _bass_exec_p,
        install_neuronx_cc_hook,
        partition_id_tensor,
    )

    install_neuronx_cc_hook()
    nc = _build_kernel(float(bl_val))

    partition_name = (
        nc.partition_id_tensor.name if nc.partition_id_tensor else None
    )
    in_names, out_names, out_avals = [], [], []
    for alloc in nc.m.functions[0].allocations:
        if not isinstance(alloc, mybir.MemoryLocationSet):
            continue
        name = alloc.memorylocations[0].name
        if alloc.kind == "ExternalInput":
            if name != partition_name:
                in_names.append(name)
        elif alloc.kind == "ExternalOutput":
            out_names.append(name)
            out_avals.append(
                jax.core.ShapedArray(
                    tuple(alloc.tensor_shape), mybir.dt.np(alloc.dtype)
                )
            )
    n_params = len(in_names)
    all_names = in_names + out_names
    if partition_name is not None:
        all_names = all_names + [partition_name]
    donate = tuple(range(n_params, n_params + len(out_names)))

    def _body(*args):
        operands = list(args)
        if partition_name is not None:
            operands.append(partition_id_tensor())
        outs = _bass_exec_p.bind(
            *operands,
            out_avals=tuple(out_avals),
            in_names=tuple(all_names),
            out_names=tuple(out_names),
            lowering_input_output_aliases=(),
            sim_require_finite=True,
            sim_require_nnan=True,
            nc=nc,
        )
        return tuple(outs)

    devices = jax.devices()[:NCORES]
    assert len(devices) == NCORES
    mesh = Mesh(np.asarray(devices), ("core",))
    sharding = NamedSharding(mesh, PartitionSpec("core"))
    sharded = jax.jit(
        shard_map(
            _body,
            mesh=mesh,
            in_specs=(PartitionSpec("core"),) * (n_params + len(out_names)),
            out_specs=(PartitionSpec("core"),) * len(out_names),
            check_rep=False,
        ),
        donate_argnums=donate,
        keep_unused=True,
    )

    _STATE = dict(
        bl_val=float(bl_val),
        nc=nc,
        sharded=sharded,
        sharding=sharding,
        in_names=in_names,
        out_avals=out_avals,
        const_dev=None,       # name -> device array (replicated weights)
        const_key=None,       # blake2b of the host weight bytes + ident scale
        prev_out=None,        # device buffer donated into the next call
    )
    return _STATE


def _const_arrays(Wf0, bf0, Wf1, bf1, Wf2, bf2, Wl, ident_scale):
    f32 = np.float32
    per_core = {
        "w0t": np.ascontiguousarray(Wf0.T),                     # (H, W)
        "w1t": np.ascontiguousarray(Wf1.T),                     # (W, W)
        "w2t": np.ascontiguousarray(Wf2.T),                     # (W, H*D)
        "wlt": np.ascontiguousarray(Wl[0][:, None]),            # (H, 1)
        "b0": np.ascontiguousarray(bf0[:, None]),
        "b1": np.ascontiguousarray(bf1[:, None]),
        "b2": np.ascontiguousarray(bf2[None, :]),
        "ones1": np.ones((1, S), f32),
        "ident": np.eye(S, dtype=f32) * f32(ident_scale),
    }
    return {k: np.concatenate([v] * NCORES, axis=0) for k, v in per_core.items()}


# ------------------------------------------------------------------- driver
def kernel(ts, cd, cc, cb, ca, Wi0, bi0, Wi1, bi1, Wi2, bi2,
           Wf0, bf0, Wf1, bf1, Wf2, bf2, Wl, bl):
    import jax
    import ml_dtypes

    f32 = np.float32
    Wf0, Wf1, Wf2, Wl = (np.asarray(x, f32) for x in (Wf0, Wf1, Wf2, Wl))
    bf0, bf1, bf2, bl = (np.asarray(x, f32) for x in (bf0, bf1, bf2, bl))

    dxq9, y0t, ident_scale = _host_precompute(
        ts, cd, cc, cb, ca, Wi0, bi0, Wi1, bi1, Wi2, bi2)

    st = _get_state(float(bl[0]))

    # device-resident replicated weights, re-verified by hash each call
    key = hashlib.blake2b(
        b"".join(x.tobytes() for x in (Wf0, bf0, Wf1, bf1, Wf2, bf2, Wl))
        + np.float64(ident_scale).tobytes(),
        digest_size=16,
    ).digest()
    if st["const_key"] != key:
        consts = _const_arrays(Wf0, bf0, Wf1, bf1, Wf2, bf2, Wl, ident_scale)
        st["const_dev"] = {
            k: jax.device_put(v, st["sharding"]) for k, v in consts.items()
        }
        st["const_key"] = key

    # per-call dynamic inputs (async puts; no host block)
    dyn = {
        "dx9": jax.device_put(dxq9, st["sharding"]),
        "y0t": jax.device_put(np.ascontiguousarray(y0t), st["sharding"]),
    }

    # donated output allocation: previous call's output buffer, else zeros
    zo = st["prev_out"]
    if zo is None:
        zo = jax.device_put(
            np.zeros((B, T), ml_dtypes.bfloat16), st["sharding"])

    args = [
        dyn[nm] if nm in dyn else st["const_dev"][nm] for nm in st["in_names"]
    ]
    (out,) = st["sharded"](*args, zo)
    st["prev_out"] = out

    return np.asarray(out).astype(f32)


# revision 9
# speedup vs baseline: 6.7274x; 1.8835x over previous
"""Neural CDE kernel for Trainium2 (8 NeuronCores, data-parallel over batch).

Problem shapes (hardcoded per contract): B=512, T=1024, D=8, H=64, W=128.

Host side (fast path, ts rows identical as produced by setup_inputs):
knot index / frac from ts row 0 (exact fp32 accumulation semantics), then a
jax-CPU jitted fused pass builds the quantized spline-derivative tensor and
the initial-MLP state:
  dxq9[b,k,d<8] = e4m3(-2*C*dt*dX[b,k,d]),  dxq9[b,k,8] = e4m3(C*dt*sum_d dX)
  with C = 2**14; the 1/C descale is folded into the transpose identity
  matrix shipped to the device (ident = I/C), so on device
  q = (1/C) * [sum_d S_d * (-2*C*dt*dX_d) + 1 * (C*dt*sum dX)]
    = dt * sum_d tanh(z_d) * dX_d        (tanh(z) = 1 - 2*sigmoid(-2z)).

Device side (per core, 64 samples, scan fully unrolled; all activations use
the natural_log_exp ACT table -- no 1.3us table reloads):
  p1 = Wf0 @ y            (PE, weight-stationary)
  h1 = ln(1 + exp(p1+b0)) (ACT Exp + Ln(bias=1))
  p2 = Wf1 @ h1           (PE)
  h2 = ln(1 + exp(p2+b1)) (ACT)
  z  = Wf2 @ h2 + b2      (PE, data-stationary, + K=1 ones-matmul for bias)
  S  = sigmoid(-2z) = exp(-ln(1+exp(2z)))          (ACT x3)
  q[s,h] = sum_d S9[s,(h,d)] * dxq9[s,k,d]         (DVE mul + grouped reduce)
           where S9 has a constant-1 column at d=8
  y += (q^T)/C            (PE transpose via scaled identity + DVE add)
  ro[:,k] = y^T @ Wl      (PE, N=1 matmul into accumulating PSUM bank)
Final: sigmoid via the same exp/ln chain, last Exp emits bf16, DMA out.

Dispatch: the shard_map jit, the compiled NEFF, and the device-resident
replicated weights are cached across calls (weights re-verified by hash each
call); per call only dxq9 (fp8, 4.7MB) and y0t (128KB) are transferred, the
previous call's output buffer is donated as the new output allocation, and
the bf16 output (1MB) is fetched back.
"""

import hashlib

import numpy as np

B, T, D, H, W = 512, 1024, 8, 64, 128
NCORES = 8
S = B // NCORES  # samples per core = 64
D9 = D + 1       # padded derivative cols
C_SCALE = float(2 ** 14)  # fp8 pre-scale; descale folded into ident input


# ----------------------------------------------------------------- host math
_FUSED_JIT = None


def _get_fused_jit():
    """jax-CPU jitted per-core gather+FMA+quantize and init-MLP passes."""
    global _FUSED_JIT
    if _FUSED_JIT is None:
        import jax
        import jax.numpy as jnp

        cpu = jax.devices("cpu")[0]

        def _dx_slice(cb, cc, cd, idx0, frac0, dtv):
            # operands are one core's (S, T-1, D) coefficient slices
            cbg = cb[:, idx0]
            ccg = cc[:, idx0]
            cdg = cd[:, idx0]
            fr = frac0[None, :, None]
            dX = cbg + fr * (2.0 * ccg + 3.0 * fr * cdg)       # (S, T, D)
            a = (-2.0 * C_SCALE * dtv) * dX
            s = (C_SCALE * dtv) * dX.sum(axis=2, keepdims=True)
            dxq9 = jnp.concatenate([a, s], axis=2)
            return dxq9.astype(jnp.float8_e4m3).reshape(S, T * D9)

        def _y0(ca0, Wi0, bi0, Wi1, bi1, Wi2, bi2):
            h = jax.nn.relu(ca0 @ Wi0.T + bi0)
            h = jax.nn.relu(h @ Wi1.T + bi1)
            y0 = h @ Wi2.T + bi2                               # (B, H)
            return y0.reshape(NCORES, S, H).transpose(0, 2, 1).reshape(B, H)

        jit_dx = jax.jit(_dx_slice)
        jit_y0 = jax.jit(_y0)

        def run_dx(*a):
            with jax.default_device(cpu):
                return jit_dx(*a)

        def run_y0(*a):
            with jax.default_device(cpu):
                return jit_y0(*a)

        _FUSED_JIT = (run_dx, run_y0)
    return _FUSED_JIT


def _row0_grid(ts):
    """Knot index / frac for the shared uniform grid (exact fp32 cumsum)."""
    f32 = np.float32
    dt = f32(ts[0, 1] - ts[0, 0])
    incs = np.concatenate([ts[0, :1], np.full(T - 1, dt, f32)])
    t0 = np.cumsum(incs, dtype=f32)
    idx0 = np.clip(np.searchsorted(ts[0], t0, side="right") - 1, 0, T - 2)
    frac0 = (t0 - ts[0][idx0]).astype(f32)
    return dt, idx0.astype(np.int32), frac0


def _host_precompute(ts, cd, cc, cb, ca, Wi0, bi0, Wi1, bi1, Wi2, bi2):
    """Returns (dxq9 (B, T*9) fp8e4m3, y0t (B, H) f32, ident_scale float).

    dxq9 rows are in batch order == concatenated per-core blocks; y0t is the
    per-core-transposed y0 ((core, H, S) flattened on axis 0).
    """
    f32 = np.float32
    ts = np.asarray(ts, f32)
    if bool((ts[1:] == ts[:1]).all()):
        # fast path: every row of ts identical (uniform grid from the oracle)
        dt, idx0, frac0 = _row0_grid(ts)
        run_dx, run_y0 = _get_fused_jit()
        cb32, cc32, cd32 = (np.asarray(x, f32) for x in (cb, cc, cd))
        parts = []
        for c in range(NCORES):
            sl = slice(c * S, (c + 1) * S)
            parts.append(np.asarray(
                run_dx(cb32[sl], cc32[sl], cd32[sl], idx0, frac0, dt)))
        dxq9 = np.concatenate(parts, axis=0)
        y0t = np.asarray(run_y0(
            np.asarray(ca, f32)[:, 0, :],
            np.asarray(Wi0, f32), np.asarray(bi0, f32),
            np.asarray(Wi1, f32), np.asarray(bi1, f32),
            np.asarray(Wi2, f32), np.asarray(bi2, f32),
        ))
        return dxq9, y0t, 1.0 / C_SCALE

    # general fallback (never taken for the oracle's inputs): per-row grids
    import ml_dtypes

    dt = (ts[:, 1] - ts[:, 0]).astype(f32)  # (B,)
    incs = np.concatenate([ts[:, :1], np.tile(dt[:, None], (1, T - 1))], axis=1)
    t0 = np.cumsum(incs, axis=1, dtype=f32)
    idx = np.empty((B, T), np.int64)
    for b in range(B):
        idx[b] = np.searchsorted(ts[b], t0[b], side="right") - 1
    idx = np.clip(idx, 0, T - 2)
    frac = (t0 - np.take_along_axis(ts, idx, axis=1)).astype(f32)
    rows = np.arange(B)[:, None]
    fr = frac[:, :, None]
    cb, cc, cd = (np.asarray(x, f32) for x in (cb, cc, cd))
    dX = (cb[rows, idx] + fr * (f32(2.0) * cc[rows, idx]
                                + f32(3.0) * fr * cd[rows, idx])).astype(f32)
    dtb = dt[:, None, None]
    a = f32(-2.0 * C_SCALE) * dtb * dX
    s = (f32(C_SCALE) * dtb[:, :, 0] * dX.sum(axis=2)).astype(f32)
    vmax = max(np.abs(a).max(), np.abs(s).max(), 1e-30)
    extra = 1.0
    while vmax / extra > 200.0:  # keep quantized values in e4m3 normal range
        extra *= 2.0
    dxq9 = np.empty((B, T, D9), ml_dtypes.float8_e4m3)
    dxq9[:, :, :D] = (a / f32(extra)).astype(ml_dtypes.float8_e4m3)
    dxq9[:, :, D] = (s / f32(extra)).astype(ml_dtypes.float8_e4m3)
    dxq9 = dxq9.reshape(B, T * D9)

    a0 = np.asarray(ca, f32)[:, 0, :]
    hh = np.maximum(a0 @ np.asarray(Wi0, f32).T + np.asarray(bi0, f32), 0)
    hh = np.maximum(hh @ np.asarray(Wi1, f32).T + np.asarray(bi1, f32), 0)
    y0 = (hh @ np.asarray(Wi2, f32).T + np.asarray(bi2, f32)).astype(f32)
    y0t = np.ascontiguousarray(
        y0.reshape(NCORES, S, H).transpose(0, 2, 1).reshape(B, H))
    return dxq9, y0t, extra / C_SCALE


# --------------------------------------------------------------- bass kernel
def _build_kernel(bl_val):
    import concourse.bass as bass
    import concourse.bacc as bacc
    import concourse.mybir as mybir
    from concourse.tile import TileContext

    f32 = mybir.dt.float32
    bf16 = mybir.dt.bfloat16
    fp8 = mybir.dt.float8e4
    AF = mybir.ActivationFunctionType
    ALU = mybir.AluOpType

    nc = bacc.Bacc("TRN2")

    # DRAM I/O (per-core shapes)
    d_w0t = nc.dram_tensor("w0t", [H, W], f32, kind="ExternalInput")      # Wf0^T
    d_w1t = nc.dram_tensor("w1t", [W, W], f32, kind="ExternalInput")      # Wf1^T
    d_w2t = nc.dram_tensor("w2t", [W, H * D], f32, kind="ExternalInput")  # Wf2^T
    d_wlt = nc.dram_tensor("wlt", [H, 1], f32, kind="ExternalInput")      # Wl^T
    d_b0 = nc.dram_tensor("b0", [W, 1], f32, kind="ExternalInput")
    d_b1 = nc.dram_tensor("b1", [W, 1], f32, kind="ExternalInput")
    d_b2 = nc.dram_tensor("b2", [1, H * D], f32, kind="ExternalInput")
    d_ones = nc.dram_tensor("ones1", [1, S], f32, kind="ExternalInput")
    d_ident = nc.dram_tensor("ident", [S, S], f32, kind="ExternalInput")  # I/C
    d_dx9 = nc.dram_tensor("dx9", [S, T * D9], fp8, kind="ExternalInput")
    d_y0t = nc.dram_tensor("y0t", [H, S], f32, kind="ExternalInput")
    d_out = nc.dram_tensor("out", [S, T], bf16, kind="ExternalOutput")

    UNROLL = 16
    assert T % UNROLL == 0

    with TileContext(nc) as tc:
        with (
            tc.tile_pool(name="const", bufs=1) as cpool,
            tc.tile_pool(name="state", bufs=1) as spool,
            tc.tile_pool(name="work", bufs=2) as wpool,
            tc.tile_pool(name="ps", bufs=2, space="PSUM") as ppool,
            tc.tile_pool(name="ps1", bufs=1, space="PSUM") as p1pool,
        ):
            # constants
            w0t = cpool.tile([H, W], f32, tag="w0t")
            w1t = cpool.tile([W, W], f32, tag="w1t")
            w2t = cpool.tile([W, H * D], f32, tag="w2t")
            wlt = cpool.tile([H, 1], f32, tag="wlt")
            b0 = cpool.tile([W, 1], f32, tag="b0")
            b1 = cpool.tile([W, 1], f32, tag="b1")
            b2 = cpool.tile([1, H * D], f32, tag="b2")
            ones1 = cpool.tile([1, S], f32, tag="ones1")
            ident = cpool.tile([S, S], f32, tag="ident")
            dx9q = cpool.tile([S, T * D9], fp8, tag="dx9q")
            for dst, src in [
                (w0t, d_w0t), (w1t, d_w1t), (w2t, d_w2t), (wlt, d_wlt),
                (b0, d_b0), (b1, d_b1), (b2, d_b2), (ones1, d_ones),
                (ident, d_ident), (dx9q, d_dx9),
            ]:
                nc.gpsimd.dma_start(dst[:], src[:])

            # fp8 -> bf16 bulk upcast (Copy is in every ACT table)
            dx9 = cpool.tile([S, T * D9], bf16, tag="dx9")
            nc.scalar.activation(dx9[:], dx9q[:], AF.Copy)

            # state
            y = spool.tile([H, S], f32, tag="y")  # (h, s)
            nc.gpsimd.dma_start(y[:], d_y0t[:])
            # S9 double buffer, const-1 column at d=8
            s9 = [
                spool.tile([S, H * D9], bf16, tag=f"s9_{i}", name=f"s9_{i}")
                for i in range(2)
            ]
            for t_ in s9:
                v = t_[:].rearrange("s (h d) -> s h d", d=D9)
                nc.vector.memset(v[:, :, D : D + 1], 1.0)

            ro_sb = spool.tile([S, T], f32, tag="ro_sb")
            ro_ps = p1pool.tile([S, UNROLL], f32, tag="ro_ps")

            # Constants settle before any compute touches them: a matmul
            # (S3_LW struct) cannot carry multiple HWDGE sem waits.
            tc.strict_bb_all_engine_barrier()

            with tc.For_i(0, T // UNROLL, 1) as iv:
              ibase = iv * (UNROLL * D9)
              for j in range(UNROLL):
                k = j  # static within the unrolled body
                s9k = s9[k % 2]
                # ---- mm1: p1 = Wf0 @ y  -> (W, S)
                p1 = ppool.tile([W, S], f32, tag="p12")
                nc.tensor.matmul(p1[:], w0t[:], y[:], start=True, stop=True)
                # ---- softplus 1 (with bias b0 folded into Exp)
                u1 = wpool.tile([W, S], f32, tag="u1")
                h1 = wpool.tile([W, S], f32, tag="h1")
                nc.scalar.activation(u1[:], p1[:], AF.Exp, bias=b0[:])
                nc.scalar.activation(h1[:], u1[:], AF.Ln, bias=1.0)
                # ---- mm2
                p2 = ppool.tile([W, S], f32, tag="p12")
                nc.tensor.matmul(p2[:], w1t[:], h1[:], start=True, stop=True)
                u2 = wpool.tile([W, S], f32, tag="u2")
                h2 = wpool.tile([W, S], f32, tag="h2")
                nc.scalar.activation(u2[:], p2[:], AF.Exp, bias=b1[:])
                nc.scalar.activation(h2[:], u2[:], AF.Ln, bias=1.0)
                # ---- mm3: z = h2^T W2T + b2 -> (S, H*D)
                vf = ppool.tile([S, H * D], f32, tag="vf")
                nc.tensor.matmul(vf[:], ones1[:], b2[:], start=True, stop=False)
                nc.tensor.matmul(vf[:], h2[:], w2t[:], start=False, stop=True)
                # ---- S = sigmoid(-2z) = exp(-ln(1+exp(2z)))
                e2 = wpool.tile([S, H * D], f32, tag="e2")
                l2 = wpool.tile([S, H * D], f32, tag="l2")
                nc.scalar.activation(e2[:], vf[:], AF.Exp, scale=2.0)
                nc.scalar.activation(l2[:], e2[:], AF.Ln, bias=1.0)
                s9v = s9k[:].rearrange("s (h d) -> s h d", d=D9)
                l2v = l2[:].rearrange("s (h d) -> s h d", d=D)
                nc.scalar.activation(s9v[:, :, 0:D], l2v, AF.Exp, scale=-1.0)
                # ---- q[s,h] = sum_d S9 * dxq9  (broadcast dx over h)
                m1 = wpool.tile([S, H * D9], bf16, tag="m1")
                dxk = dx9[:, bass.ds(ibase + j * D9, D9)]
                dxb = dxk.rearrange("s (o d) -> s o d", o=1)
                m1v = m1[:].rearrange("s (h d) -> s h d", d=D9)
                s9vv = s9k[:].rearrange("s (h d) -> s h d", d=D9)
                in0b, in1b = bass.broadcast_tensor_aps(s9vv, dxb)
                nc.vector.tensor_tensor(m1v, in0b, in1b, ALU.mult)
                q = wpool.tile([S, H], f32, tag="q")
                nc.vector.tensor_reduce(
                    q[:], m1v, axis=mybir.AxisListType.X, op=ALU.add
                )
                # ---- y += (q^T)/C: real matmul q^T @ (I/C) — transpose mode
                # would ignore the identity's values, dropping the descale
                qt = ppool.tile([H, S], f32, tag="qt")
                nc.tensor.matmul(qt[:], q[:], ident[:], start=True, stop=True)
                nc.vector.tensor_tensor(y[:], y[:], qt[:], ALU.add)
                # ---- readout column
                nc.tensor.matmul(
                    ro_ps[:, j : j + 1], y[:], wlt[:], start=True, stop=True
                )
                if j == UNROLL - 1:
                    nc.vector.tensor_copy(
                        ro_sb[:, bass.ds(iv * UNROLL, UNROLL)], ro_ps[:]
                    )

            # ---- final sigmoid(v + bl) = exp(-ln(1+exp(-v-bl)))
            eo = spool.tile([S, T], f32, tag="eo")
            eo16 = spool.tile([S, T], bf16, tag="eo16")
            nc.scalar.activation(eo[:], ro_sb[:], AF.Exp, scale=-1.0,
                                 bias=float(-bl_val))
            nc.scalar.activation(eo[:], eo[:], AF.Ln, bias=1.0)
            nc.scalar.activation(eo16[:], eo[:], AF.Exp, scale=-1.0)
            nc.sync.dma_start(d_out[:], eo16[:])

    nc.compile()
    return nc


# ------------------------------------------------------------------ dispatch
_STATE = None
LAST_RESULTS = None  # kept for test harness compatibility (always None)


def _get_state(bl_val):
    """Build-once state: bass module, shard_map jit, mesh, name order."""
    global _STATE
    if _STATE is not None and _STATE["bl_val"] == float(bl_val):
        return _STATE

    import jax
    from jax.sharding import Mesh, NamedSharding, PartitionSpec
    from jax.experimental.shard_map import shard_map
    import concourse.mybir as mybir
    from concourse.bass2jax import (
        _bass_exec_p,
        install_neuronx_cc_hook,
        partition_id_tensor,
    )

    install_neuronx_cc_hook()
    nc = _build_kernel(float(bl_val))

    partition_name = (
        nc.partition_id_tensor.name if nc.partition_id_tensor else None
    )
    in_names, out_names, out_avals = [], [], []
    for alloc in nc.m.functions[0].allocations:
        if not isinstance(alloc, mybir.MemoryLocationSet):
            continue
        name = alloc.memorylocations[0].name
        if alloc.kind == "ExternalInput":
            if name != partition_name:
                in_names.append(name)
        elif alloc.kind == "ExternalOutput":
            out_names.append(name)
            out_avals.append(
                jax.core.ShapedArray(
                    tuple(alloc.tensor_shape), mybir.dt.np(alloc.dtype)
                )
            )
    n_params = len(in_names)
    all_names = in_names + out_names
    if partition_name is not None:
        all_names = all_names + [partition_name]
    donate = tuple(range(n_params, n_params + len(out_names)))

    def _body(*args):
        operands = list(args)
        if partition_name is not None:
            operands.append(partition_id_tensor())
        outs = _bass_exec_p.bind(
            *operands,
            out_avals=tuple(out_avals),
            in_names=tuple(all_names),
            out_names=tuple(out_names),
            lowering_input_output_aliases=(),
            sim_require_finite=True,
            sim_require_nnan=True,
            nc=nc,
        )
        return tuple(outs)

    devices = jax.devices()[:NCORES]
    assert len(devices) == NCORES
    mesh = Mesh(np.asarray(devices), ("core",))
    sharding = NamedSharding(mesh, PartitionSpec("core"))
    sharded = jax.jit(
        shard_map(
            _body,
            mesh=mesh,
            in_specs=(PartitionSpec("core"),) * (n_params + len(out_names)),
            out_specs=(PartitionSpec("core"),) * len(out_names),
            check_rep=False,
        ),
        donate_argnums=donate,
        keep_unused=True,
    )

    _STATE = dict(
        bl_val=float(bl_val),
        nc=nc,
        sharded=sharded,
        sharding=sharding,
        devices=list(devices),
        in_names=in_names,
        out_avals=out_avals,
        const_dev=None,       # name -> device array (replicated weights)
        const_key=None,       # blake2b of the host weight bytes + ident scale
        prev_out=None,        # device buffer donated into the next call
    )
    return _STATE


def _const_arrays(Wf0, bf0, Wf1, bf1, Wf2, bf2, Wl, ident_scale):
    f32 = np.float32
    per_core = {
        "w0t": np.ascontiguousarray(Wf0.T),                     # (H, W)
        "w1t": np.ascontiguousarray(Wf1.T),                     # (W, W)
        "w2t": np.ascontiguousarray(Wf2.T),                     # (W, H*D)
        "wlt": np.ascontiguousarray(Wl[0][:, None]),            # (H, 1)
        "b0": np.ascontiguousarray(bf0[:, None]),
        "b1": np.ascontiguousarray(bf1[:, None]),
        "b2": np.ascontiguousarray(bf2[None, :]),
        "ones1": np.ones((1, S), f32),
        "ident": np.eye(S, dtype=f32) * f32(ident_scale),
    }
    return {k: np.concatenate([v] * NCORES, axis=0) for k, v in per_core.items()}


# ------------------------------------------------------------------- driver
def kernel(ts, cd, cc, cb, ca, Wi0, bi0, Wi1, bi1, Wi2, bi2,
           Wf0, bf0, Wf1, bf1, Wf2, bf2, Wl, bl):
    import jax
    import ml_dtypes

    f32 = np.float32
    Wf0, Wf1, Wf2, Wl = (np.asarray(x, f32) for x in (Wf0, Wf1, Wf2, Wl))
    bf0, bf1, bf2, bl = (np.asarray(x, f32) for x in (bf0, bf1, bf2, bl))

    st = _get_state(float(bl[0]))

    ts32 = np.asarray(ts, f32)
    if bool((ts32[1:] == ts32[:1]).all()):
        # fast path: compute each core's dX slice, put it to that device
        # immediately so the tunnel transfer overlaps the remaining compute
        dt, idx0, frac0 = _row0_grid(ts32)
        run_dx, run_y0 = _get_fused_jit()
        cb32, cc32, cd32 = (np.asarray(x, f32) for x in (cb, cc, cd))
        bufs = []
        for c in range(NCORES):
            sl = slice(c * S, (c + 1) * S)
            dxc = np.asarray(
                run_dx(cb32[sl], cc32[sl], cd32[sl], idx0, frac0, dt))
            bufs.append(jax.device_put(dxc, st["devices"][c]))
        dx_dev = jax.make_array_from_single_device_arrays(
            (B, T * D9), st["sharding"], bufs)
        y0t = np.asarray(run_y0(
            np.asarray(ca, f32)[:, 0, :],
            np.asarray(Wi0, f32), np.asarray(bi0, f32),
            np.asarray(Wi1, f32), np.asarray(bi1, f32),
            np.asarray(Wi2, f32), np.asarray(bi2, f32),
        ))
        ident_scale = 1.0 / C_SCALE
    else:
        dxq9, y0t, ident_scale = _host_precompute(
            ts, cd, cc, cb, ca, Wi0, bi0, Wi1, bi1, Wi2, bi2)
        dx_dev = jax.device_put(dxq9, st["sharding"])

    # device-resident replicated weights, re-verified by hash each call
    key = hashlib.blake2b(
        b"".join(x.tobytes() for x in (Wf0, bf0, Wf1, bf1, Wf2, bf2, Wl))
        + np.float64(ident_scale).tobytes(),
        digest_size=16,
    ).digest()
    if st["const_key"] != key:
        consts = _const_arrays(Wf0, bf0, Wf1, bf1, Wf2, bf2, Wl, ident_scale)
        st["const_dev"] = {
            k: jax.device_put(v, st["sharding"]) for k, v in consts.items()
        }
        st["const_key"] = key

    # per-call dynamic inputs (async puts; no host block)
    dyn = {
        "dx9": dx_dev,
        "y0t": jax.device_put(np.ascontiguousarray(y0t), st["sharding"]),
    }

    # donated output allocation: previous call's output buffer, else zeros
    zo = st["prev_out"]
    if zo is None:
        zo = jax.device_put(
            np.zeros((B, T), ml_dtypes.bfloat16), st["sharding"])

    args = [
        dyn[nm] if nm in dyn else st["const_dev"][nm] for nm in st["in_names"]
    ]
    (out,) = st["sharded"](*args, zo)
    st["prev_out"] = out

    return np.asarray(out).astype(f32)


# revision 11
# speedup vs baseline: 9.1577x; 1.3613x over previous
"""Neural CDE kernel for Trainium2 (8 NeuronCores, data-parallel over batch).

Problem shapes (hardcoded per contract): B=512, T=1024, D=8, H=64, W=128.

Host side (fast path, ts rows identical as produced by setup_inputs):
knot index / frac from ts row 0 (exact fp32 accumulation semantics), then a
jax-CPU jitted fused pass builds the quantized spline-derivative tensor
  dxq8[b,k,d] = e4m3(-2*C*dt*dX[b,k,d]),  C = 2**14
per core, each put to its device as soon as computed so the (CPU-bound
zstd) tunnel transfer overlaps the remaining compute.  The 1/C descale is
folded into the identity matrix shipped in the const pack, so on device
  q = (1/C) * [sum_d S_d * (-2*C*dt*dX_d) + 1 * (C*dt*sum_d dX_d)]
    = dt * sum_d tanh(z_d) * dX_d        (tanh(z) = 1 - 2*sigmoid(-2z)).

Device side (per core, 64 samples, scan fully unrolled; all activations use
the natural_log_exp ACT table -- no 1.3us table reloads):
  prologue: upcast dxq8 fp8->bf16 into a 9-col-strided tile, build col 8 as
  -0.5 * sum_d cols (= C*dt*sum dX), and run the initial MLP
  y0 = Wi2 @ relu(Wi1 @ relu(Wi0 @ a0 + bi0) + bi1) + bi2 from a0 = ca[:,0].
  per step:
    p1 = Wf0 @ y            (PE, weight-stationary)
    h1 = ln(1 + exp(p1+b0)) (ACT Exp + Ln(bias=1))
    p2 = Wf1 @ h1           (PE)
    h2 = ln(1 + exp(p2+b1)) (ACT)
    z  = Wf2 @ h2 + b2      (PE, data-stationary, + K=1 ones-matmul for bias)
    S  = sigmoid(-2z) = exp(-ln(1+exp(2z)))          (ACT x3)
    q[s,h] = sum_d S9[s,(h,d)] * dx9[s,k,d]          (DVE mul + grouped reduce)
             where S9 has a constant-1 column at d=8
    y += q^T @ (I/C)        (PE matmul vs scaled identity + DVE add)
    ro[:,k] = y^T @ Wl      (PE, N=1 matmul into accumulating PSUM bank)
  Final: sigmoid via the same exp/ln chain, last Exp emits bf16, DMA out.

Dispatch: the shard_map jit, the compiled NEFF, and the device-resident
replicated const pack are cached across calls (re-verified by hash each
call); per call only dxq8 (fp8, 4.2MB) and a0t (16KB) are transferred, the
previous call's output buffer is donated as the new output allocation, and
the bf16 output (1MB) is fetched back.
"""

import hashlib

import numpy as np

B, T, D, H, W = 512, 1024, 8, 64, 128
NCORES = 8
S = B // NCORES  # samples per core = 64
D9 = D + 1       # padded derivative cols
C_SCALE = float(2 ** 14)  # fp8 pre-scale; descale folded into ident values

# const-pack column layout ([128, PACK_COLS] f32 per core)
_PK_W1T = (0, 128)        # Wf1^T  [128, 128]
_PK_W2T = (128, 640)      # Wf2^T  [128, 512]
_PK_WI1 = (640, 768)      # Wi1^T  [128, 128]
_PK_WI2 = (768, 832)      # Wi2^T  [128, 64]
_PK_B0 = (832, 833)       # bf0    [128, 1]
_PK_B1 = (833, 834)       # bf1    [128, 1]
_PK_BI0 = (834, 835)      # bi0    [128, 1]
_PK_BI1 = (835, 836)      # bi1    [128, 1]
_PK_W0T = (836, 964)      # Wf0^T  [64, 128]   (partitions 0:64)
_PK_IDENT = (964, 1028)   # I/C    [64, 64]    (partitions 0:64)
_PK_WLT = (1028, 1029)    # Wl^T   [64, 1]     (partitions 0:64)
_PK_BI2 = (1029, 1030)    # bi2    [64, 1]     (partitions 0:64)
_PK_WI0 = (1030, 1158)    # Wi0^T  [8, 128]    (partitions 0:8)
_PK_B2 = (1158, 1670)     # bf2    [1, 512]    (partition 0)
_PK_ONES = (1670, 1734)   # ones   [1, 64]     (partition 0)
PACK_COLS = 1734


# ----------------------------------------------------------------- host math
_FUSED_JIT = None


def _get_fused_jit():
    """jax-CPU jitted per-core gather+FMA+quantize pass."""
    global _FUSED_JIT
    if _FUSED_JIT is None:
        import jax
        import jax.numpy as jnp

        cpu = jax.devices("cpu")[0]

        def _dx_slice(cb, cc, cd, idx0, frac0, dtv):
            # operands are one core's (S, T-1, D) coefficient slices
            cbg = cb[:, idx0]
            ccg = cc[:, idx0]
            cdg = cd[:, idx0]
            fr = frac0[None, :, None]
            dX = cbg + fr * (2.0 * ccg + 3.0 * fr * cdg)       # (S, T, D)
            a = (-2.0 * C_SCALE * dtv) * dX
            return a.astype(jnp.float8_e4m3).reshape(S, T * D)

        jit_dx = jax.jit(_dx_slice)

        def run_dx(*a):
            with jax.default_device(cpu):
                return jit_dx(*a)

        _FUSED_JIT = run_dx
    return _FUSED_JIT


def _row0_grid(ts):
    """Knot index / frac for the shared uniform grid (exact fp32 cumsum)."""
    f32 = np.float32
    dt = f32(ts[0, 1] - ts[0, 0])
    incs = np.concatenate([ts[0, :1], np.full(T - 1, dt, f32)])
    t0 = np.cumsum(incs, dtype=f32)
    idx0 = np.clip(np.searchsorted(ts[0], t0, side="right") - 1, 0, T - 2)
    frac0 = (t0 - ts[0][idx0]).astype(f32)
    return dt, idx0.astype(np.int32), frac0


def _host_precompute(ts, cd, cc, cb, ca, Wi0, bi0, Wi1, bi1, Wi2, bi2):
    """Returns (dxq8 (B, T*8) fp8e4m3, a0t (NCORES*D, S) f32, ident_scale).

    dxq8 rows are in batch order == concatenated per-core blocks; a0t is the
    per-core-transposed ca[:, 0, :] ((core, D, S) flattened on axis 0).
    """
    f32 = np.float32
    ts = np.asarray(ts, f32)
    a0 = np.asarray(ca, f32)[:, 0, :]                          # (B, D)
    a0t = np.ascontiguousarray(
        a0.reshape(NCORES, S, D).transpose(0, 2, 1).reshape(NCORES * D, S))
    if bool((ts[1:] == ts[:1]).all()):
        # fast path: every row of ts identical (uniform grid from the oracle)
        dt, idx0, frac0 = _row0_grid(ts)
        run_dx = _get_fused_jit()
        cb32, cc32, cd32 = (np.asarray(x, f32) for x in (cb, cc, cd))
        parts = []
        for c in range(NCORES):
            sl = slice(c * S, (c + 1) * S)
            parts.append(np.asarray(
                run_dx(cb32[sl], cc32[sl], cd32[sl], idx0, frac0, dt)))
        return np.concatenate(parts, axis=0), a0t, 1.0 / C_SCALE

    # general fallback (never taken for the oracle's inputs): per-row grids
    import ml_dtypes

    dt = (ts[:, 1] - ts[:, 0]).astype(f32)  # (B,)
    incs = np.concatenate([ts[:, :1], np.tile(dt[:, None], (1, T - 1))], axis=1)
    t0 = np.cumsum(incs, axis=1, dtype=f32)
    idx = np.empty((B, T), np.int64)
    for b in range(B):
        idx[b] = np.searchsorted(ts[b], t0[b], side="right") - 1
    idx = np.clip(idx, 0, T - 2)
    frac = (t0 - np.take_along_axis(ts, idx, axis=1)).astype(f32)
    rows = np.arange(B)[:, None]
    fr = frac[:, :, None]
    cb, cc, cd = (np.asarray(x, f32) for x in (cb, cc, cd))
    dX = (cb[rows, idx] + fr * (f32(2.0) * cc[rows, idx]
                                + f32(3.0) * fr * cd[rows, idx])).astype(f32)
    a = f32(-2.0 * C_SCALE) * dt[:, None, None] * dX
    vmax = max(np.abs(a).max(), 1e-30)
    extra = 1.0
    while vmax / extra > 200.0:  # keep quantized values in e4m3 normal range
        extra *= 2.0
    dxq8 = (a / f32(extra)).astype(ml_dtypes.float8_e4m3).reshape(B, T * D)
    return dxq8, a0t, extra / C_SCALE


# --------------------------------------------------------------- bass kernel
def _build_kernel(bl_val):
    import concourse.bass as bass
    import concourse.bacc as bacc
    import concourse.mybir as mybir
    from concourse.tile import TileContext

    f32 = mybir.dt.float32
    bf16 = mybir.dt.bfloat16
    fp8 = mybir.dt.float8e4
    AF = mybir.ActivationFunctionType
    ALU = mybir.AluOpType

    nc = bacc.Bacc("TRN2")

    d_pack = nc.dram_tensor("pack", [128, PACK_COLS], f32, kind="ExternalInput")
    d_dx8 = nc.dram_tensor("dx8", [S, T * D], fp8, kind="ExternalInput")
    d_a0t = nc.dram_tensor("a0t", [D, S], f32, kind="ExternalInput")
    d_out = nc.dram_tensor("out", [S, T], bf16, kind="ExternalOutput")

    UNROLL = 16
    assert T % UNROLL == 0

    with TileContext(nc) as tc:
        with (
            tc.tile_pool(name="const", bufs=1) as cpool,
            tc.tile_pool(name="state", bufs=1) as spool,
            tc.tile_pool(name="work", bufs=2) as wpool,
            tc.tile_pool(name="ps", bufs=2, space="PSUM") as ppool,
            tc.tile_pool(name="ps1", bufs=1, space="PSUM") as p1pool,
        ):
            pack = cpool.tile([128, PACK_COLS], f32, tag="pack")
            dx8q = cpool.tile([S, T * D], fp8, tag="dx8q")
            a0t = cpool.tile([D, S], f32, tag="a0t")
            nc.gpsimd.dma_start(pack[:], d_pack[:])
            nc.gpsimd.dma_start(dx8q[:], d_dx8[:])
            nc.gpsimd.dma_start(a0t[:], d_a0t[:])

            pk = pack[:]
            w1t = pk[:, _PK_W1T[0]:_PK_W1T[1]]
            w2t = pk[:, _PK_W2T[0]:_PK_W2T[1]]
            wi1t = pk[:, _PK_WI1[0]:_PK_WI1[1]]
            wi2t = pk[:, _PK_WI2[0]:_PK_WI2[1]]
            b0 = pk[:, _PK_B0[0]:_PK_B0[1]]
            b1 = pk[:, _PK_B1[0]:_PK_B1[1]]
            bi0 = pk[:, _PK_BI0[0]:_PK_BI0[1]]
            bi1 = pk[:, _PK_BI1[0]:_PK_BI1[1]]
            w0t = pk[0:H, _PK_W0T[0]:_PK_W0T[1]]
            ident = pk[0:S, _PK_IDENT[0]:_PK_IDENT[1]]
            wlt = pk[0:H, _PK_WLT[0]:_PK_WLT[1]]
            bi2 = pk[0:H, _PK_BI2[0]:_PK_BI2[1]]
            wi0t = pk[0:D, _PK_WI0[0]:_PK_WI0[1]]
            b2 = pk[0:1, _PK_B2[0]:_PK_B2[1]]
            ones1 = pk[0:1, _PK_ONES[0]:_PK_ONES[1]]

            # dx9: bf16, 9-col stride; cols 0:8 upcast from fp8, col 8 built
            # on device as -0.5 * sum_d cols == C*dt*sum_d dX
            dx9 = spool.tile([S, T * D9], bf16, tag="dx9")
            colsum = spool.tile([S, T], f32, tag="colsum")

            # S9 double buffer, const-1 column at d=8
            s9 = [
                spool.tile([S, H * D9], bf16, tag=f"s9_{i}", name=f"s9_{i}")
                for i in range(2)
            ]
            for t_ in s9:
                v = t_[:].rearrange("s (h d) -> s h d", d=D9)
                nc.vector.memset(v[:, :, D : D + 1], 1.0)

            y = spool.tile([H, S], f32, tag="y")  # (h, s)
            ro_sb = spool.tile([S, T], f32, tag="ro_sb")
            ro_ps = p1pool.tile([S, UNROLL], f32, tag="ro_ps")

            # Constants settle before any compute touches them: a matmul
            # (S3_LW struct) cannot carry multiple HWDGE sem waits.
            tc.strict_bb_all_engine_barrier()

            # ---- prologue: dx9 assembly
            dx9v = dx9[:].rearrange("s (t d) -> s t d", d=D9)
            dx8v = dx8q[:].rearrange("s (t d) -> s t d", d=D)
            nc.scalar.activation(dx9v[:, :, 0:D], dx8v, AF.Copy)
            nc.vector.tensor_reduce(
                colsum[:], dx9v[:, :, 0:D], axis=mybir.AxisListType.X,
                op=ALU.add,
            )
            csv = colsum[:].rearrange("s (t o) -> s t o", o=1)
            nc.scalar.activation(dx9v[:, :, D:D9], csv, AF.Copy, scale=-0.5)

            # ---- prologue: initial MLP y0 from a0
            hp = ppool.tile([W, S], f32, tag="p12")
            nc.tensor.matmul(hp[:], wi0t, a0t[:], start=True, stop=True)
            h1s = wpool.tile([W, S], f32, tag="u1")
            nc.scalar.activation(h1s[:], hp[:], AF.Relu, bias=bi0)
            hp2 = ppool.tile([W, S], f32, tag="p12")
            nc.tensor.matmul(hp2[:], wi1t, h1s[:], start=True, stop=True)
            h2s = wpool.tile([W, S], f32, tag="u2")
            nc.scalar.activation(h2s[:], hp2[:], AF.Relu, bias=bi1)
            yp = ppool.tile([H, S], f32, tag="qt")
            nc.tensor.matmul(yp[:], wi2t, h2s[:], start=True, stop=True)
            nc.scalar.activation(y[:], yp[:], AF.Identity, bias=bi2)

            with tc.For_i(0, T // UNROLL, 1) as iv:
              ibase = iv * (UNROLL * D9)
              for j in range(UNROLL):
                s9k = s9[j % 2]
                # ---- mm1: p1 = Wf0 @ y  -> (W, S)
                p1 = ppool.tile([W, S], f32, tag="p12")
                nc.tensor.matmul(p1[:], w0t, y[:], start=True, stop=True)
                # ---- softplus 1 (with bias b0 folded into Exp)
                u1 = wpool.tile([W, S], f32, tag="u1")
                h1 = wpool.tile([W, S], f32, tag="h1")
                nc.scalar.activation(u1[:], p1[:], AF.Exp, bias=b0)
                nc.scalar.activation(h1[:], u1[:], AF.Ln, bias=1.0)
                # ---- mm2
                p2 = ppool.tile([W, S], f32, tag="p12")
                nc.tensor.matmul(p2[:], w1t, h1[:], start=True, stop=True)
                u2 = wpool.tile([W, S], f32, tag="u2")
                h2 = wpool.tile([W, S], f32, tag="h2")
                nc.scalar.activation(u2[:], p2[:], AF.Exp, bias=b1)
                nc.scalar.activation(h2[:], u2[:], AF.Ln, bias=1.0)
                # ---- mm3: z = h2^T W2T + b2 -> (S, H*D)
                vf = ppool.tile([S, H * D], f32, tag="vf")
                nc.tensor.matmul(vf[:], ones1, b2, start=True, stop=False)
                nc.tensor.matmul(vf[:], h2[:], w2t, start=False, stop=True)
                # ---- S = sigmoid(-2z) = exp(-ln(1+exp(2z)))
                e2 = wpool.tile([S, H * D], f32, tag="e2")
                l2 = wpool.tile([S, H * D], f32, tag="l2")
                nc.scalar.activation(e2[:], vf[:], AF.Exp, scale=2.0)
                nc.scalar.activation(l2[:], e2[:], AF.Ln, bias=1.0)
                s9v = s9k[:].rearrange("s (h d) -> s h d", d=D9)
                l2v = l2[:].rearrange("s (h d) -> s h d", d=D)
                nc.scalar.activation(s9v[:, :, 0:D], l2v, AF.Exp, scale=-1.0)
                # ---- q[s,h] = sum_d S9 * dx9  (broadcast dx over h)
                m1 = wpool.tile([S, H * D9], bf16, tag="m1")
                dxk = dx9[:, bass.ds(ibase + j * D9, D9)]
                dxb = dxk.rearrange("s (o d) -> s o d", o=1)
                m1v = m1[:].rearrange("s (h d) -> s h d", d=D9)
                s9vv = s9k[:].rearrange("s (h d) -> s h d", d=D9)
                in0b, in1b = bass.broadcast_tensor_aps(s9vv, dxb)
                nc.vector.tensor_tensor(m1v, in0b, in1b, ALU.mult)
                q = wpool.tile([S, H], f32, tag="q")
                nc.vector.tensor_reduce(
                    q[:], m1v, axis=mybir.AxisListType.X, op=ALU.add
                )
                # ---- y += (q^T)/C: real matmul q^T @ (I/C) — transpose mode
                # would ignore the identity's values, dropping the descale
                qt = ppool.tile([H, S], f32, tag="qt")
                nc.tensor.matmul(qt[:], q[:], ident, start=True, stop=True)
                nc.vector.tensor_tensor(y[:], y[:], qt[:], ALU.add)
                # ---- readout column
                nc.tensor.matmul(
                    ro_ps[:, j : j + 1], y[:], wlt, start=True, stop=True
                )
                if j == UNROLL - 1:
                    nc.vector.tensor_copy(
                        ro_sb[:, bass.ds(iv * UNROLL, UNROLL)], ro_ps[:]
                    )

            # ---- final sigmoid(v + bl) = exp(-ln(1+exp(-v-bl)))
            eo = spool.tile([S, T], f32, tag="eo")
            eo16 = spool.tile([S, T], bf16, tag="eo16")
            nc.scalar.activation(eo[:], ro_sb[:], AF.Exp, scale=-1.0,
                                 bias=float(-bl_val))
            nc.scalar.activation(eo[:], eo[:], AF.Ln, bias=1.0)
            nc.scalar.activation(eo16[:], eo[:], AF.Exp, scale=-1.0)
            nc.sync.dma_start(d_out[:], eo16[:])

    nc.compile()
    return nc


# ------------------------------------------------------------------ dispatch
_STATE = None
LAST_RESULTS = None  # kept for test harness compatibility (always None)


def _get_state(bl_val):
    """Build-once state: bass module, shard_map jit, mesh, name order."""
    global _STATE
    if _STATE is not None and _STATE["bl_val"] == float(bl_val):
        return _STATE

    import jax
    from jax.sharding import Mesh, NamedSharding, PartitionSpec
    from jax.experimental.shard_map import shard_map
    import concourse.mybir as mybir
    from concourse.bass2jax import (
        _bass_exec_p,
        install_neuronx_cc_hook,
        partition_id_tensor,
    )

    install_neuronx_cc_hook()
    nc = _build_kernel(float(bl_val))

    partition_name = (
        nc.partition_id_tensor.name if nc.partition_id_tensor else None
    )
    in_names, out_names, out_avals = [], [], []
    for alloc in nc.m.functions[0].allocations:
        if not isinstance(alloc, mybir.MemoryLocationSet):
            continue
        name = alloc.memorylocations[0].name
        if alloc.kind == "ExternalInput":
            if name != partition_name:
                in_names.append(name)
        elif alloc.kind == "ExternalOutput":
            out_names.append(name)
            out_avals.append(
                jax.core.ShapedArray(
                    tuple(alloc.tensor_shape), mybir.dt.np(alloc.dtype)
                )
            )
    n_params = len(in_names)
    all_names = in_names + out_names
    if partition_name is not None:
        all_names = all_names + [partition_name]
    donate = tuple(range(n_params, n_params + len(out_names)))

    def _body(*args):
        operands = list(args)
        if partition_name is not None:
            operands.append(partition_id_tensor())
        outs = _bass_exec_p.bind(
            *operands,
            out_avals=tuple(out_avals),
            in_names=tuple(all_names),
            out_names=tuple(out_names),
            lowering_input_output_aliases=(),
            sim_require_finite=True,
            sim_require_nnan=True,
            nc=nc,
        )
        return tuple(outs)

    devices = jax.devices()[:NCORES]
    assert len(devices) == NCORES
    mesh = Mesh(np.asarray(devices), ("core",))
    sharding = NamedSharding(mesh, PartitionSpec("core"))
    sharded = jax.jit(
        shard_map(
            _body,
            mesh=mesh,
            in_specs=(PartitionSpec("core"),) * (n_params + len(out_names)),
            out_specs=(PartitionSpec("core"),) * len(out_names),
            check_rep=False,
        ),
        donate_argnums=donate,
        keep_unused=True,
    )

    _STATE = dict(
        bl_val=float(bl_val),
        nc=nc,
        sharded=sharded,
        sharding=sharding,
        devices=list(devices),
        in_names=in_names,
        out_avals=out_avals,
        const_dev=None,       # name -> device array (replicated const pack)
        const_key=None,       # blake2b of the host weight bytes + ident scale
        prev_out=None,        # device buffer donated into the next call
    )
    return _STATE


def _const_pack(Wf0, bf0, Wf1, bf1, Wf2, bf2, Wl,
                Wi0, bi0, Wi1, bi1, Wi2, bi2, ident_scale):
    f32 = np.float32
    p = np.zeros((128, PACK_COLS), f32)
    p[:, _PK_W1T[0]:_PK_W1T[1]] = Wf1.T
    p[:, _PK_W2T[0]:_PK_W2T[1]] = Wf2.T
    p[:, _PK_WI1[0]:_PK_WI1[1]] = Wi1.T
    p[:, _PK_WI2[0]:_PK_WI2[1]] = Wi2.T
    p[:, _PK_B0[0]] = bf0
    p[:, _PK_B1[0]] = bf1
    p[:, _PK_BI0[0]] = bi0
    p[:, _PK_BI1[0]] = bi1
    p[0:H, _PK_W0T[0]:_PK_W0T[1]] = Wf0.T
    p[0:S, _PK_IDENT[0]:_PK_IDENT[1]] = np.eye(S, dtype=f32) * f32(ident_scale)
    p[0:H, _PK_WLT[0]] = Wl[0]
    p[0:H, _PK_BI2[0]] = bi2
    p[0:D, _PK_WI0[0]:_PK_WI0[1]] = Wi0.T
    p[0, _PK_B2[0]:_PK_B2[1]] = bf2
    p[0, _PK_ONES[0]:_PK_ONES[1]] = 1.0
    return np.concatenate([p] * NCORES, axis=0)  # (NCORES*128, PACK_COLS)


# ------------------------------------------------------------------- driver
def kernel(ts, cd, cc, cb, ca, Wi0, bi0, Wi1, bi1, Wi2, bi2,
           Wf0, bf0, Wf1, bf1, Wf2, bf2, Wl, bl):
    import jax
    import ml_dtypes

    f32 = np.float32
    Wf0, Wf1, Wf2, Wl = (np.asarray(x, f32) for x in (Wf0, Wf1, Wf2, Wl))
    bf0, bf1, bf2, bl = (np.asarray(x, f32) for x in (bf0, bf1, bf2, bl))
    Wi0, Wi1, Wi2 = (np.asarray(x, f32) for x in (Wi0, Wi1, Wi2))
    bi0, bi1, bi2 = (np.asarray(x, f32) for x in (bi0, bi1, bi2))

    st = _get_state(float(bl[0]))

    ts32 = np.asarray(ts, f32)
    if bool((ts32[1:] == ts32[:1]).all()):
        # fast path: compute each core's dX slice, put it to that device
        # immediately so the tunnel transfer overlaps the remaining compute
        dt, idx0, frac0 = _row0_grid(ts32)
        run_dx = _get_fused_jit()
        cb32, cc32, cd32 = (np.asarray(x, f32) for x in (cb, cc, cd))
        bufs = []
        for c in range(NCORES):
            sl = slice(c * S, (c + 1) * S)
            dxc = np.asarray(
                run_dx(cb32[sl], cc32[sl], cd32[sl], idx0, frac0, dt))
            bufs.append(jax.device_put(dxc, st["devices"][c]))
        dx_dev = jax.make_array_from_single_device_arrays(
            (B, T * D), st["sharding"], bufs)
        a0 = np.asarray(ca, f32)[:, 0, :]
        a0t = np.ascontiguousarray(
            a0.reshape(NCORES, S, D).transpose(0, 2, 1).reshape(NCORES * D, S))
        ident_scale = 1.0 / C_SCALE
    else:
        dxq8, a0t, ident_scale = _host_precompute(
            ts, cd, cc, cb, ca, Wi0, bi0, Wi1, bi1, Wi2, bi2)
        dx_dev = jax.device_put(dxq8, st["sharding"])

    # device-resident replicated const pack, re-verified by hash each call
    key = hashlib.blake2b(
        b"".join(x.tobytes() for x in (Wf0, bf0, Wf1, bf1, Wf2, bf2, Wl,
                                       Wi0, bi0, Wi1, bi1, Wi2, bi2))
        + np.float64(ident_scale).tobytes(),
        digest_size=16,
    ).digest()
    if st["const_key"] != key:
        pack = _const_pack(Wf0, bf0, Wf1, bf1, Wf2, bf2, Wl,
                           Wi0, bi0, Wi1, bi1, Wi2, bi2, ident_scale)
        st["const_dev"] = {"pack": jax.device_put(pack, st["sharding"])}
        st["const_key"] = key

    dyn = {
        "dx8": dx_dev,
        "a0t": jax.device_put(a0t, st["sharding"]),
    }

    # donated output allocation: previous call's output buffer, else zeros
    zo = st["prev_out"]
    if zo is None:
        zo = jax.device_put(
            np.zeros((B, T), ml_dtypes.bfloat16), st["sharding"])

    args = [
        dyn[nm] if nm in dyn else st["const_dev"][nm] for nm in st["in_names"]
    ]
    (out,) = st["sharded"](*args, zo)
    st["prev_out"] = out

    return np.asarray(out).astype(f32)
